# revision 13
# baseline (speedup 1.0000x reference)
import sys

sys.path.insert(0, "/opt/trn_rl_repo")
import numpy as np
from concourse import bacc, tile
import concourse.mybir as mybir
from concourse.bass_utils import run_bass_kernel_spmd

f32 = mybir.dt.float32
f32r = mybir.dt.float32r

OUT, IN = 4096, 4096
B, S = 4, 2048
T = B * S                      # 8192 tokens
TG, OG = 2, 4                  # 2 token groups x 4 out-feature groups = 8 cores
T_CORE = T // TG               # 4096
O_CORE = OUT // OG             # 1024
KS = IN // 128                 # 32 contraction slabs
TC = T_CORE // 128             # 32 token chunks per core
N_CORES = 8

_NC_CACHE = {}
LAST_RESULT = None


def _build_nc():
    nc = bacc.Bacc("TRN2", target_bir_lowering=False, debug=False,
                   num_devices=N_CORES)
    xT_d = nc.dram_tensor("xT", [IN, T_CORE], f32, kind="ExternalInput").ap()
    wT_d = nc.dram_tensor("wT", [IN, O_CORE], f32, kind="ExternalInput").ap()
    bias_d = nc.dram_tensor("bias", [128, O_CORE], f32,
                            kind="ExternalInput").ap()
    out_d = nc.dram_tensor("out", [T_CORE, O_CORE], f32,
                           kind="ExternalOutput").ap()

    WARM = 4                 # chunks processed slab-major while weights load
    GT = WARM * 128          # 512 warm-up tokens

    with tile.TileContext(nc) as tc:
        with (
            tc.tile_pool(name="wres", bufs=1) as wres,
            tc.tile_pool(name="xp", bufs=2) as xp,
            tc.tile_pool(name="op", bufs=2) as op,
            tc.tile_pool(name="cst", bufs=1) as cst,
            tc.tile_pool(name="ps", bufs=1, space="PSUM") as ps,
        ):
            bias_t = cst.tile([128, O_CORE], f32)

            pp = [ps.tile([128, 512], f32, tag=f"pp{i}", name=f"pp{i}")
                  for i in range(8)]
            wts = [wres.tile([128, O_CORE], f32r, tag=f"wt{k}", name=f"wt{k}")
                   for k in range(KS)]

            def evict(c, pA, pB):
                ot = op.tile([128, O_CORE], f32, tag="ot", name="ot")
                nc.vector.tensor_tensor(ot[:, 0:512], pA[:],
                                        bias_t[:, 0:512],
                                        op=mybir.AluOpType.add)
                nc.vector.tensor_tensor(ot[:, 512:O_CORE], pB[:],
                                        bias_t[:, 512:O_CORE],
                                        op=mybir.AluOpType.add)
                nc.scalar.dma_start(out_d[c * 128:(c + 1) * 128, :], ot[:])

            # Warm-up: stream w^T slabs in on three DMA queues (sync: o-half0,
            # scalar/ACT: o-half1, gpsimd: x tokens), interleaved with
            # slab-major matmuls of the first WARM chunks so the PE consumes
            # each slab as soon as it lands.
            for ks in range(KS):
                r = slice(ks * 128, (ks + 1) * 128)
                xts = xp.tile([128, GT], f32r, tag="xts", bufs=3, name="xts")
                if ks == 0:
                    # Split slab 0 across both HWDGE queues and land the
                    # first 128 tokens early so the first matmul's three
                    # dependencies all arrive ~0.4us sooner.
                    nc.sync.dma_start(wts[0][:, 0:256],
                                      wT_d[r, 0:256].bitcast(f32r))
                    nc.scalar.dma_start(wts[0][:, 256:512],
                                        wT_d[r, 256:512].bitcast(f32r))
                    nc.gpsimd.dma_start(xts[:, 0:128],
                                        xT_d[r, 0:128].bitcast(f32r))
                    nc.sync.dma_start(wts[0][:, 512:768],
                                      wT_d[r, 512:768].bitcast(f32r))
                    nc.scalar.dma_start(wts[0][:, 768:O_CORE],
                                        wT_d[r, 768:O_CORE].bitcast(f32r))
                    nc.gpsimd.dma_start(xts[:, 128:GT],
                                        xT_d[r, 128:GT].bitcast(f32r))
                else:
                    nc.sync.dma_start(wts[ks][:, 0:512],
                                      wT_d[r, 0:512].bitcast(f32r))
                    nc.scalar.dma_start(wts[ks][:, 512:O_CORE],
                                        wT_d[r, 512:O_CORE].bitcast(f32r))
                    nc.gpsimd.dma_start(xts[:], xT_d[r, 0:GT].bitcast(f32r))
                for c in range(WARM):
                    lhs = xts[:, c * 128:(c + 1) * 128]
                    nc.tensor.matmul(pp[2 * c][:], lhs, wts[ks][:, 0:512],
                                     start=(ks == 0), stop=(ks == KS - 1))
                    nc.tensor.matmul(pp[2 * c + 1][:], lhs,
                                     wts[ks][:, 512:O_CORE],
                                     start=(ks == 0), stop=(ks == KS - 1))
            nc.gpsimd.dma_start(bias_t[:], bias_d)
            for c in range(WARM):
                evict(c, pp[2 * c], pp[2 * c + 1])

            # Steady state: chunk-major, PSUM ping-pong via pp[0..3].
            for c in range(WARM, TC):
                xt = xp.tile([128, KS, 128], f32r, tag="xt", name="xt")
                nc.sync.dma_start(
                    xt[:],
                    xT_d[:, c * 128:(c + 1) * 128]
                    .rearrange("(ks p) t -> p ks t", p=128)
                    .bitcast(f32r))
                pA, pB = (pp[0], pp[1]) if c % 2 == 0 else (pp[2], pp[3])
                last = c == TC - 1
                if not last:
                    for ks in range(KS):
                        nc.tensor.matmul(pA[:], xt[:, ks, :],
                                         wts[ks][:, 0:512],
                                         start=(ks == 0), stop=(ks == KS - 1))
                        nc.tensor.matmul(pB[:], xt[:, ks, :],
                                         wts[ks][:, 512:O_CORE],
                                         start=(ks == 0), stop=(ks == KS - 1))
                    evict(c, pA, pB)
                else:
                    # Final chunk, half-major: evict o-half0 while o-half1
                    # matmuls run; the exposed o-half1 evict is split into
                    # two 256-wide halves on parallel engines/queues.
                    row = slice(c * 128, (c + 1) * 128)
                    for ks in range(KS):
                        nc.tensor.matmul(pA[:], xt[:, ks, :],
                                         wts[ks][:, 0:512],
                                         start=(ks == 0), stop=(ks == KS - 1))
                    otA = op.tile([128, 512], f32, tag="otA", name="otA")
                    nc.vector.tensor_tensor(otA[:], pA[:], bias_t[:, 0:512],
                                            op=mybir.AluOpType.add)
                    nc.scalar.dma_start(out_d[row, 0:512], otA[:])
                    for ks in range(KS):
                        nc.tensor.matmul(pB[:], xt[:, ks, :],
                                         wts[ks][:, 512:O_CORE],
                                         start=(ks == 0), stop=(ks == KS - 1))
                    otB = op.tile([128, 512], f32, tag="otB", name="otB")
                    nc.vector.tensor_tensor(otB[:], pB[:],
                                            bias_t[:, 512:O_CORE],
                                            op=mybir.AluOpType.add)
                    nc.scalar.dma_start(out_d[row, 512:768], otB[:, 0:256])
                    nc.sync.dma_start(out_d[row, 768:O_CORE], otB[:, 256:512])
    nc.finalize()
    return nc


def kernel(x, weight_high, weight_medium, weight_low,
           high_precision_mask, medium_precision_mask, low_scale, bias):
    global LAST_RESULT
    if "nc" not in _NC_CACHE:
        _NC_CACHE["nc"] = _build_nc()
    nc = _NC_CACHE["nc"]

    xT = np.ascontiguousarray(
        x.reshape(T, IN).T.astype(np.float32, copy=False))
    low_mask = ~(high_precision_mask | medium_precision_mask)
    # Same f32 ops as the reference: one rounding for the low-tier product,
    # exact adds (tier supports are disjoint).
    w = (weight_high.astype(np.float32, copy=False)
         + weight_medium.astype(np.float32)
         + low_mask * (weight_low.astype(np.float32)
                       * np.float32(low_scale[0])))
    wT = np.ascontiguousarray(w.T)
    bias = bias.astype(np.float32, copy=False)

    in_maps = []
    for core in range(N_CORES):
        tg, og = divmod(core, OG)
        in_maps.append(dict(
            xT=np.ascontiguousarray(xT[:, tg * T_CORE:(tg + 1) * T_CORE]),
            wT=np.ascontiguousarray(wT[:, og * O_CORE:(og + 1) * O_CORE]),
            bias=np.tile(bias[og * O_CORE:(og + 1) * O_CORE], (128, 1)),
        ))

    res = run_bass_kernel_spmd(nc, in_maps, core_ids=list(range(N_CORES)))
    LAST_RESULT = res

    full = np.empty((T, OUT), dtype=np.float32)
    for core in range(N_CORES):
        tg, og = divmod(core, OG)
        full[tg * T_CORE:(tg + 1) * T_CORE,
             og * O_CORE:(og + 1) * O_CORE] = res.results[core]["out"]
    return full.reshape(B, S, OUT)
